# revision 8
# baseline (speedup 1.0000x reference)
"""Trainium2 Bass kernel for the EvolvedLoss elementwise program.

Math (per element):
    m2 = o - t
    m3 = m2*m2
    m4 = tanh(c2*m2 + c22)
    m5 = m3 + c3*m4
    e  = exp(-c4*m3)
    d  = 1 + c6*m3
    loss = (e/d + c7) * m5

Key restructurings for TRN2:
  * `targets` negated on the host so m2 = o + (-t) is computed by the DMA
    itself (SWDGE accumulate-add): zero compute passes.
  * tanh via the algebraic identity tanh(z) = (q-1)/(q+1) with
    q = exp(-2z) (exact for all z in fp32; sign handled implicitly), so the
    only ACT table set needed is natural_log_exp_and_others.
  * the division e/d via exponentials: e/d = exp(-c4*m3 - ln(1+c6*m3)).
    (Custom DVE ops and the reciprocal ACT table are unusable with this
    toolchain; DVE's iterative reciprocal measures ~15 cyc/elem.)

Engine plan per [128, F] tile (per core: N_PP=32768 elem/partition):
    HWDGE-SP   : o in (1 MiB per transfer)
    SWDGE/Pool : -t in, accumulated onto o's tile -> X = m2 in SBUF
    Pool       : m3 = X*X (tensor_tensor mult)
    ACT (one natural_log_exp table load):
        q   = Exp(-2c2*X - 2c22)
        L1  = Ln(q + 1)
        inv = Exp(-L1)                 # 1/(1+q)
        L2  = Ln(c6*m3 + 1)
        er  = Exp(-w)                  # = exp(-c4*m3)/(1+c6*m3) = e/d
    DVE        :
        m4n = (q - 1) * inv            # = -tanh(z)   (scalar_tensor_tensor)
        w   = c4*m3 + L2               # (scalar_tensor_tensor)
        loss= (er + c7) * m5           # (scalar_tensor_tensor, m5 in PSUM)
    PE         : m5 = I @ m3 + (-c3 I) @ m4n  -> PSUM
    HWDGE-ACT  : loss out

Post-pass _split_waits() adapts the Tile-scheduled module to this
neuronxcc build (max one sync-wait per instruction; no
EVENT_SEMAPHORE_RANGE_CLEAR).
"""

import os

import numpy as np

B, D = 4096, 8192
N_CORES = 8
ROWS_PER_CORE = B // N_CORES          # 512
P = 128
N_PP = ROWS_PER_CORE * D // P         # 32768 elements per partition per core
F = 2048                              # tile free-dim width
N_TILES = N_PP // F
MM = 512                              # matmul free-dim chunk (one PSUM bank)

_cache = {}


def _split_waits(nc):
    """Make the scheduled module acceptable to this neuronxcc build:

    1. No instruction may carry more than one sync wait -> move extra waits
       onto standalone EventSemaphore instructions just before it (same
       engine, program order == identical semantics).
    2. EVENT_SEMAPHORE_RANGE_CLEAR (opcode 176) is rejected by codegen ->
       replace with per-sem sem-sub-imm EventSemaphores that subtract each
       sem's statically-known final value (the program is straight-line, so
       totals are exact), restoring the zero state for re-execution.
    """
    import concourse.mybir as mybir

    net = {}
    for fn in nc.m.functions:
        for bb in fn.blocks:
            for inst in bb.instructions:
                si = inst.sync_info
                if not si or not si.on_update:
                    continue
                for u in si.on_update:
                    if u.sync_type != "semaphore" or u.update_value is None:
                        continue
                    sign = -1 if u.update_mode in ("sem-dec", "sem-sub-imm") else 1
                    key = int(u.id)
                    net[key] = net.get(key, 0) + sign * int(u.update_value)

    for fn in nc.m.functions:
        for bb in fn.blocks:
            new = []
            changed = False
            for inst in bb.instructions:
                if (
                    type(inst).__name__ == "InstISA"
                    and getattr(inst, "isa_opcode", None) == 176
                ):
                    changed = True
                    d = dict(inst.ant_dict)
                    for sem_id in range(d["range_first"], d["range_last"] + 1):
                        amt = net.get(sem_id, 0)
                        if amt == 0:
                            continue
                        es = mybir.InstEventSemaphore(
                            name=f"{inst.name}_clr{sem_id}", engine=inst.engine
                        )
                        es.sync_info = mybir.SyncInfo(
                            on_wait=[],
                            on_update=[
                                mybir.SyncUpdate(
                                    sync_type="semaphore",
                                    id=sem_id,
                                    update_mode="sem-sub-imm",
                                    update_value=amt,
                                )
                            ],
                        )
                        new.append(es)
                    continue
                si = inst.sync_info
                waits = list(si.on_wait) if si and si.on_wait else []
                if len(waits) > 1 and inst.engine is not None:
                    changed = True
                    for j, w in enumerate(waits[:-1]):
                        es = mybir.InstEventSemaphore(
                            name=f"{inst.name}_presync{j}", engine=inst.engine
                        )
                        es.sync_info = mybir.SyncInfo(on_wait=[w], on_update=[])
                        new.append(es)
                    inst.sync_info = mybir.SyncInfo(
                        on_wait=[waits[-1]], on_update=list(si.on_update or [])
                    )
                new.append(inst)
            if changed:
                bb.instructions = new
    return nc


def _build(c: np.ndarray, c2: np.ndarray):
    """Trace the Bass program with constants baked in. Returns nc."""
    import concourse.bass as bass
    import concourse.mybir as mybir
    from concourse import tile

    f32 = mybir.dt.float32
    AF = mybir.ActivationFunctionType
    OP = mybir.AluOpType

    c2_, c22_ = float(c[2]), float(c2[2])
    c3_, c4_, c6_, c7_ = float(c[3]), float(c[4]), float(c[6]), float(c[7])

    nc = bass.Bass(
        "TRN2",
        target_bir_lowering=False,
        debug=False,
        enable_asserts=False,
        num_devices=N_CORES,
    )
    o_d = nc.dram_tensor("o", [P, N_PP], f32, kind="ExternalInput").ap()
    tn_d = nc.dram_tensor("tn", [P, N_PP], f32, kind="ExternalInput").ap()
    loss_d = nc.dram_tensor("loss", [P, N_PP], f32, kind="ExternalOutput").ap()

    with tile.TileContext(nc) as tc:
        with (
            tc.tile_pool(name="cpool", bufs=1) as cpool,
            tc.tile_pool(name="io", bufs=3) as iop,
            tc.tile_pool(name="tmp", bufs=2) as tmp,
        ):
            q_bias = cpool.tile([P, 1], f32)
            nc.gpsimd.memset(q_bias[:], -2.0 * c22_)

            for i in range(N_TILES):
                sl = slice(i * F, (i + 1) * F)
                x = iop.tile([P, F], f32)          # becomes m2 = o - t
                nc.sync.dma_start(x[:], o_d[:, sl])
                nc.gpsimd.dma_start(x[:], tn_d[:, sl], accum_op=OP.add)

                m3 = tmp.tile([P, F], f32)
                nc.gpsimd.tensor_tensor(m3[:], x[:], x[:], OP.mult)

                # tanh branch: q = exp(-2z), z = c2*x + c22
                # tanh(z) = (1-q)/(1+q);  m4n = (q-1)/(1+q) = -tanh(z)
                q = tmp.tile([P, F], f32)
                nc.scalar.activation(q[:], x[:], AF.Exp, bias=q_bias[:], scale=-2.0 * c2_)
                L1 = tmp.tile([P, F], f32)
                nc.scalar.activation(L1[:], q[:], AF.Ln, bias=1.0, scale=1.0)
                inv = tmp.tile([P, F], f32)
                nc.scalar.activation(inv[:], L1[:], AF.Exp, scale=-1.0)
                m4n = tmp.tile([P, F], f32)
                nc.vector.scalar_tensor_tensor(
                    m4n[:], q[:], 1.0, inv[:], OP.subtract, OP.mult
                )

                # m5 = m3 + c3*tanh(z) = m3 - c3*m4n
                m5 = tmp.tile([P, F], f32)
                nc.vector.scalar_tensor_tensor(
                    m5[:], m4n[:], -c3_, m3[:], OP.mult, OP.add
                )

                # e/d = exp(-c4*m3 - ln(1+c6*m3))
                L2 = tmp.tile([P, F], f32)
                nc.scalar.activation(L2[:], m3[:], AF.Ln, bias=1.0, scale=c6_)
                w = tmp.tile([P, F], f32)
                nc.vector.scalar_tensor_tensor(
                    w[:], m3[:], c4_, L2[:], OP.mult, OP.add
                )
                er = tmp.tile([P, F], f32)
                nc.scalar.activation(er[:], w[:], AF.Exp, scale=-1.0)

                out = iop.tile([P, F], f32)
                nc.vector.scalar_tensor_tensor(
                    out[:], er[:], c7_, m5[:], OP.add, OP.mult
                )
                nc.sync.dma_start(loss_d[:, sl], out[:])

    return _split_waits(nc)


def make_in_maps(outputs: np.ndarray, neg_targets: np.ndarray):
    in_maps = []
    for i in range(N_CORES):
        rs = slice(i * ROWS_PER_CORE, (i + 1) * ROWS_PER_CORE)
        in_maps.append(
            {
                "o": np.ascontiguousarray(outputs[rs]).reshape(P, N_PP),
                "tn": np.ascontiguousarray(neg_targets[rs]).reshape(P, N_PP),
            }
        )
    return in_maps


def get_nc(constants: np.ndarray, constants_2: np.ndarray):
    c = np.asarray(constants, dtype=np.float32)
    c2 = np.asarray(constants_2, dtype=np.float32)
    key = (c.tobytes(), c2.tobytes())
    if key not in _cache:
        _cache[key] = _build(c, c2)
    return _cache[key]


def kernel(outputs, targets, constants, constants_2):
    from concourse import bass_utils

    outputs = np.asarray(outputs, dtype=np.float32)
    neg_targets = -np.asarray(targets, dtype=np.float32)
    nc = get_nc(constants, constants_2)
    in_maps = make_in_maps(outputs, neg_targets)
    res = bass_utils.run_bass_kernel_spmd(nc, in_maps, core_ids=list(range(N_CORES)))
    full = np.empty((B, D), dtype=np.float32)
    for i in range(N_CORES):
        full[i * ROWS_PER_CORE : (i + 1) * ROWS_PER_CORE] = (
            res.results[i]["loss"].reshape(ROWS_PER_CORE, D)
        )
    return full
